# revision 10
# baseline (speedup 1.0000x reference)
"""Distributed Trainium2 Bass kernel for single-head attention with
softmax over the QUERY axis (faithful to the reference).

Reference math (per batch b):
    q = x @ Wq + bq          # [S, D]   S=4096, D=48
    k = x @ Wk + bk
    v = x @ Wv + bv
    s = (q @ k.T) / sqrt(D)  # [S_q, S_k]
    p = softmax(s, axis=QUERY)          # normalize each k-COLUMN over q
    out = p @ v              # [S_q, D]

Sharding (k-split): 8 cores = 4 batches x 2 KEY-halves. Core c handles
batch c//2 and key rows [ (c%2)*2048, (c%2+1)*2048 ), for ALL 4096
queries. The softmax denominator colsum[k] = sum_q exp(s[q,k]) is then
fully LOCAL (free-axis accumulation inside the exp instructions) -- no
mid-stream collectives. The only collective is one bf16 ReduceScatter
of the output partials at the end; a tiny dummy collective at t=0
absorbs the cold CC-stream setup.

Layout: everything is computed TRANSPOSED on chip.
  - scores_T[k, q] tiles have k on partitions / q on the free axis, so
    colsum[k] falls out of the exp accum_out (the ACCUMULATOR_READ is
    pipelined behind the next ACTIVATE -- measured free).
  - Q and K are projected with weights DUPLICATED at PE columns 64-111,
    so the row-packing replicas (partitions 64-111) come out of the
    matmul itself -- zero replica DMAs, one-op epilogues.
  - The exp stream: first half sweeps q in two 1024-wide passes over
    all 16 k-tiles (q-major, DMA friendly); second half goes k-major
    (both remaining q-sweeps per tile back to back) so each tile's
    colsum completes immediately and its attention matmuls follow one
    slot later, inside the exp stream.
  - The per-column normalization folds into V (vs[k,:] = v[k,:]/colsum);
    V^T -> V transposes ride the DMA xbar engine.
  - Projections borrow the attention accumulator's PSUM banks before
    attention starts.

SPMD note: one NEFF for all 8 cores, so every AP offset is shared.
Queries live in GLOBAL positions; only the key half differs, via a
host-sliced second input (xtk = the core's own x^T chunks).

bq is dropped: softmax over q is invariant to per-k constant shifts,
and bq only contributes bq.(x_k Wk + bk), constant along q.

exp() runs without max-subtraction: scores*scale is N(0,~1/9), bounded
by ~|2.5| for these inputs, so exp stays well inside fp32 range.
"""

import sys

for _p in ("/opt/trn_rl_repo",):
    if _p not in sys.path:
        sys.path.insert(0, _p)

import numpy as np
import ml_dtypes

import concourse.bass as bass
import concourse.tile as tile
from concourse import bacc, mybir
from concourse.bass_utils import run_bass_kernel_spmd

N_CORES = 8
B = 4
S = 4096
DIM = 768
D = 48
SH = S // 2          # key rows per core / q rows per RS shard
P = 128
NKT = SH // P        # 16 local k-tiles
NC = DIM // P        # 6 contraction tiles for projections
QC = 512             # q chunk = one PSUM bank / projection granularity
NQC = S // QC        # 8 q chunks
SCALE = 1.0 / np.sqrt(np.float32(D))
RG = [[0, 1], [2, 3], [4, 5], [6, 7]]
N_WARM_MM = 26       # PE warmup matmuls during the first DMA wait

BF16 = mybir.dt.bfloat16
F32 = mybir.dt.float32


def _build():
    nc = bacc.Bacc(
        "TRN2",
        target_bir_lowering=False,
        debug=False,
        num_devices=N_CORES,
    )

    # x^T chunks packed host-side as [chunk][partition][ci][512] so each
    # chunk DMA is 128 descriptors x 6KB (near-peak HBM read).
    xtq_d = nc.dram_tensor("xtq", [NQC, P, NC, QC], BF16, kind="ExternalInput")
    xtk_d = nc.dram_tensor("xtk", [4, P, NC, QC], BF16, kind="ExternalInput")
    # Q / K duplicated at cols 64-111 (row-packing replicas for free).
    wqq_d = nc.dram_tensor("wqq", [P, NC, 112], BF16, kind="ExternalInput")
    wkk_d = nc.dram_tensor("wkk", [P, NC, 112], BF16, kind="ExternalInput")
    wv_d = nc.dram_tensor("wv", [P, NC, 112], BF16, kind="ExternalInput")
    # col 0: K bias duplicated (bk,0,bk); col 1: V bias (0,0,bv)
    bias_d = nc.dram_tensor("bias", [112, 2], F32, kind="ExternalInput")
    out_d = nc.dram_tensor("out", [D, SH], BF16, kind="ExternalOutput")

    with tile.TileContext(nc) as tc:
        with (
            tc.tile_pool(name="consts", bufs=1) as consts,
            tc.tile_pool(name="big", bufs=1) as big,
            tc.tile_pool(name="xtqp", bufs=3) as xtqp,
            tc.tile_pool(name="xtkp", bufs=2) as xtkp,
            tc.tile_pool(name="pop", bufs=1, space="PSUM") as pop,
            tc.tile_pool(name="scp", bufs=2, space="PSUM") as scp,
            tc.tile_pool(name="dram", bufs=1, space="DRAM") as dram,
        ):
            # ---- constants (wqq first: it feeds the PE warmup) -------
            wqq_sb = consts.tile([P, NC, 112], BF16, tag="wqq")
            nc.sync.dma_start(out=wqq_sb, in_=wqq_d[:, :, :])
            wkk_sb = consts.tile([P, NC, 112], BF16, tag="wkk")
            nc.sync.dma_start(out=wkk_sb, in_=wkk_d[:, :, :])
            wv_sb = consts.tile([P, NC, 112], BF16, tag="wv")
            nc.sync.dma_start(out=wv_sb, in_=wv_d[:, :, :])
            bias_sb = consts.tile([112, 2], F32, tag="bias")
            nc.sync.dma_start(out=bias_sb, in_=bias_d[:, :])

            # vT rows 112:127 are junk fed to the xbar transpose; zero
            # them so v_sb cols 48:63 are defined (never read). On the
            # DVE queue BEFORE any epilogue: the gpsimd queue is frozen
            # by the collective prelude barrier for the first ~70us.
            vT = big.tile([P, SH], BF16, tag="vT")       # V^T @64-111
            nc.vector.memset(vT[96:P, :], 0)

            # ---- CC-stream warmup: tiny ReduceScatter ----------------
            warm_in = dram.tile([112, 2], F32, tag="warm_in")
            warm_out = dram.tile([112, 1], F32, tag="warm_out")
            nc.gpsimd.dma_start(out=warm_in, in_=bias_d[:, :])
            nc.gpsimd.collective_compute(
                "ReduceScatter",
                mybir.AluOpType.add,
                replica_groups=RG,
                ins=[warm_in.opt()],
                outs=[warm_out.opt()],
            )

            # ---- persistent SBUF tensors -----------------------------
            qT = big.tile([112, S], BF16, tag="qT")      # Q^T @0-47 and @64-111
            kkT = big.tile([112, SH], BF16, tag="kkT")   # K^T @0-47 and @64-111
            v_sb = big.tile([P, NKT, 64], BF16, tag="v")     # V [k, d] tiles
            vs_sb = big.tile([P, NKT, D], BF16, tag="vs")    # V / colsum
            e_sb = big.tile([P, NKT, S], BF16, tag="e")      # exp(scores^T)
            csh = big.tile([P, NKT, 4], F32, tag="csh")      # colsum partials
            cs = big.tile([P, NKT], F32, tag="cs")
            rec = big.tile([P, NKT], F32, tag="rec")
            out_sb = big.tile([P, 4, QC], BF16, tag="out")

            # attention accumulator: 4 banks, column-packed
            #   parts 0:48  bank b = global q-chunk b     (q 0..2047)
            #   parts 64:112 bank b = global q-chunk 4+b  (q 2048..4095)
            # projections borrow bank slots 0/1 BEFORE attention starts.
            po = pop.tile([P, 4, QC], F32, tag="po")

            # ---- PE warmup: HAM to 8/8 during the first DMA wait -----
            warm_ps = scp.tile([P, 2, QC], F32, tag="sct")
            for _ in range(N_WARM_MM):
                nc.tensor.matmul(
                    warm_ps[0:112, 0, 0:112], wqq_sb[:, 0, :], wqq_sb[:, 0, :],
                    start=True, stop=True, skip_group_check=True,
                )

            # ---- projections into rotating po bank slots -------------
            qdma = {}
            kdma = {}
            slot_rr = [0]

            def q_dma(c, split=False):
                t = xtqp.tile([P, NC, QC], BF16, tag="xtq")
                if split:
                    nc.sync.dma_start(out=t[:, 0:3, :], in_=xtq_d[c][:, 0:3, :])
                    nc.sync.dma_start(out=t[:, 3:6, :], in_=xtq_d[c][:, 3:6, :])
                else:
                    nc.sync.dma_start(out=t, in_=xtq_d[c])
                qdma[c] = t

            def k_dma(j, split=False):
                t = xtkp.tile([P, NC, QC], BF16, tag="xtk")
                if split:
                    nc.sync.dma_start(out=t[:, 0:3, :], in_=xtk_d[j][:, 0:3, :])
                    nc.sync.dma_start(out=t[:, 3:6, :], in_=xtk_d[j][:, 3:6, :])
                else:
                    nc.sync.dma_start(out=t, in_=xtk_d[j])
                kdma[j] = t

            def _proj(t, w_sb, dst, bias_ap):
                sl = slot_rr[0]
                slot_rr[0] ^= 1
                pp = po[0:112, sl, :]
                for ci in range(NC):
                    nc.tensor.matmul(
                        pp, w_sb[:, ci, :], t[:, ci, :],
                        start=(ci == 0), stop=(ci == NC - 1),
                        skip_group_check=True,
                    )
                if bias_ap is None:
                    nc.vector.tensor_copy(out=dst, in_=pp)
                else:
                    nc.vector.tensor_scalar(
                        out=dst, in0=pp, scalar1=bias_ap, scalar2=None,
                        op0=mybir.AluOpType.add,
                    )

            def q_proj(c):
                t = qdma[c]
                qsl = slice(c * QC, (c + 1) * QC)
                _proj(t, wqq_sb, qT[0:112, qsl], None)
                qdma.pop(c)

            def k_proj(j):
                t = kdma[j]
                ksl = slice(j * QC, (j + 1) * QC)
                _proj(t, wkk_sb, kkT[0:112, ksl], bias_sb[0:112, 0:1])

            def v_proj(j):
                t = kdma.pop(j)
                ksl = slice(j * QC, (j + 1) * QC)
                sl = slot_rr[0]
                slot_rr[0] ^= 1
                pp = po[0:112, sl, :]
                for ci in range(NC):
                    nc.tensor.matmul(
                        pp, wv_sb[:, ci, :], t[:, ci, :],
                        start=(ci == 0), stop=(ci == NC - 1),
                        skip_group_check=True,
                    )
                nc.vector.tensor_scalar(
                    out=vT[64:112, ksl], in0=pp[64:112, :],
                    scalar1=bias_sb[64:112, 1:2], scalar2=None,
                    op0=mybir.AluOpType.add,
                )

            def v_tr(kt):
                # V^T [64,128] slice -> v_sb [128,64] via DMA xbar
                nc.sync.dma_start_transpose(
                    out=v_sb[:, kt, :],
                    in_=vT[64:P, kt * P:(kt + 1) * P],
                )

            # ---- scores + exp for one (k-tile, sweep) unit -----------
            def unit(kt, sw):
                sct = scp.tile([P, 2, QC], F32, tag="sct")
                ksl = slice(kt * P, (kt + 1) * P)
                nc.tensor.matmul(
                    sct[:, 0, :],
                    kkT[0:D, ksl],
                    qT[0:D, (2 * sw) * QC:(2 * sw + 1) * QC],
                    start=True, stop=True,
                    tile_position=(0, 0), skip_group_check=True,
                )
                nc.tensor.matmul(
                    sct[:, 1, :],
                    kkT[64:64 + D, ksl],
                    qT[64:64 + D, (2 * sw + 1) * QC:(2 * sw + 2) * QC],
                    start=True, stop=True,
                    tile_position=(64, 0), skip_group_check=True,
                )
                nc.scalar.activation(
                    out=e_sb[:, kt, sw * 2 * QC:(sw + 1) * 2 * QC],
                    in_=sct[:, :, :],
                    func=mybir.ActivationFunctionType.Exp,
                    scale=float(SCALE),
                    accum_out=csh[:, kt, sw:sw + 1],
                )

            # ---- colsum finish + V scaling for one k-tile ------------
            def finish(kt):
                nc.vector.tensor_reduce(
                    out=cs[:, kt:kt + 1], in_=csh[:, kt, :],
                    axis=mybir.AxisListType.X, op=mybir.AluOpType.add,
                )
                nc.vector.reciprocal(out=rec[:, kt:kt + 1], in_=cs[:, kt:kt + 1])
                nc.vector.tensor_scalar(
                    out=vs_sb[:, kt, :], in0=v_sb[:, kt, 0:D],
                    scalar1=rec[:, kt:kt + 1], scalar2=None,
                    op0=mybir.AluOpType.mult,
                )

            # ---- attention pair-halves (column-packed) ---------------
            def attn(kt, half):
                first = kt == 0
                last = kt == NKT - 1
                for p in (0, 1) if half == 0 else (2, 3):
                    nc.tensor.matmul(
                        po[0:D, p, :],
                        vs_sb[:, kt, :],
                        e_sb[:, kt, p * QC:(p + 1) * QC],
                        start=first, stop=last,
                        tile_position=(0, 0), skip_group_check=True,
                    )
                    pb = (p + 1) % 4
                    nc.tensor.matmul(
                        po[64:64 + D, pb, :],
                        vs_sb[:, kt, :],
                        e_sb[:, kt, (4 + pb) * QC:(5 + pb) * QC],
                        start=first, stop=last,
                        tile_position=(0, 64), skip_group_check=True,
                    )

            # ================= emission schedule ======================
            # Front: chunk 0 of K + Q chunks 0,1 gate the first unit.
            k_dma(0, split=True)
            q_dma(0, split=True)
            q_dma(1, split=True)
            k_proj(0)
            q_proj(0)
            q_proj(1)
            k_dma(1)
            q_dma(2)

            # Fill work interleaved into the first-half exp stream.
            # Input DMA issues all precede the v_tr transposes on the
            # sync queue so the (slow) transpose issues never head-block
            # a chunk load.
            fill = [
                lambda: k_proj(1),       # k chunk 1 ready before slot 4
                lambda: v_proj(0),       # frees xtk buf 0
                lambda: k_dma(2),
                lambda: q_dma(3),
                lambda: q_proj(2),
                lambda: k_proj(2),       # ready before slot 8
                lambda: v_proj(1),       # frees xtk buf 1
                lambda: k_dma(3),
                lambda: q_proj(3),
                lambda: k_proj(3),       # ready before slot 12
                lambda: q_dma(4),
                lambda: v_proj(2),
                lambda: q_proj(4),
                lambda: q_dma(5),
                lambda: v_proj(3),
                lambda: q_proj(5),
                lambda: q_dma(6),
                lambda: q_proj(6),
                lambda: q_dma(7),
                lambda: q_proj(7),
            ] + [(lambda kt=kt: v_tr(kt)) for kt in range(NKT)]
            fi = [0]

            def pop_fill(n):
                for _ in range(n):
                    if fi[0] < len(fill):
                        fill[fi[0]]()
                        fi[0] += 1

            # First half: sweeps 0,1 q-major.
            for sw in range(2):
                for kt in range(NKT):
                    unit(kt, sw)
                    pop_fill(1 if (sw == 0 or kt < 4) else 2)

            # Second half: k-major, attention one tile behind.
            for kt in range(NKT):
                unit(kt, 2)
                if kt > 0:
                    finish(kt - 1)
                    attn(kt - 1, 0)
                unit(kt, 3)
                if kt > 0:
                    attn(kt - 1, 1)
            finish(NKT - 1)
            attn(NKT - 1, 0)
            attn(NKT - 1, 1)

            # ---- epilogue: PSUM -> SBUF(bf16) -> DRAM -> RS ----------
            nc.vector.tensor_copy(out=out_sb[:, 0:2, :], in_=po[:, 0:2, :])
            nc.scalar.copy(out=out_sb[:, 2:4, :], in_=po[:, 2:4, :])

            cc_in = dram.tile([2, D, SH], BF16, tag="cc_in")
            cc_out = dram.tile([D, SH], BF16, tag="cc_out")
            nc.sync.dma_start(
                out=cc_in[0],
                in_=out_sb[0:D, :, :].rearrange("p a b -> p (a b)"),
            )
            nc.scalar.dma_start(
                out=cc_in[1],
                in_=out_sb[64:64 + D, :, :].rearrange("p a b -> p (a b)"),
            )
            nc.gpsimd.collective_compute(
                "ReduceScatter",
                mybir.AluOpType.add,
                replica_groups=RG,
                ins=[cc_in.opt()],
                outs=[cc_out.opt()],
            )
            nc.sync.dma_start(out=out_d[:, :], in_=cc_out)

    nc.compile()
    return nc


_NC_CACHE = None


def _get_nc():
    global _NC_CACHE
    if _NC_CACHE is None:
        _NC_CACHE = _build()
    return _NC_CACHE


def kernel(x, Wq, bq, Wk, bk, Wv, bv):
    x = np.asarray(x, np.float32)
    bf = ml_dtypes.bfloat16

    def pack_dup(W):
        """[768, 48] -> [128, 6, 112] bf16 with cols duplicated @64."""
        full = np.zeros((DIM, 112), np.float32)
        full[:, 0:D] = W
        full[:, 64:64 + D] = W
        return np.ascontiguousarray(
            full.reshape(NC, P, 112).transpose(1, 0, 2)
        ).astype(bf)

    wqq_h = pack_dup(np.asarray(Wq, np.float32))
    wkk_h = pack_dup(np.asarray(Wk, np.float32))
    wv_full = np.zeros((DIM, 112), np.float32)
    wv_full[:, 64:64 + D] = np.asarray(Wv, np.float32)
    wv_h = np.ascontiguousarray(
        wv_full.reshape(NC, P, 112).transpose(1, 0, 2)
    ).astype(bf)
    bias_h = np.zeros((112, 2), np.float32)
    bias_h[0:D, 0] = np.asarray(bk, np.float32).ravel()
    bias_h[64:64 + D, 0] = np.asarray(bk, np.float32).ravel()
    bias_h[64:64 + D, 1] = np.asarray(bv, np.float32).ravel()
    # bq is mathematically irrelevant: softmax over the query axis is
    # invariant to per-key constant shifts.

    w_maps = {"wqq": wqq_h, "wkk": wkk_h, "wv": wv_h, "bias": bias_h}

    in_maps = []
    chunks_by_batch = []
    for b_idx in range(B):
        xT = np.ascontiguousarray(x[b_idx].T)                  # [768, 4096]
        chunks = np.ascontiguousarray(
            xT.reshape(NC, P, NQC, QC).transpose(2, 1, 0, 3)
        ).astype(bf)                                           # [8,128,6,512]
        chunks_by_batch.append(chunks)
    for core in range(N_CORES):
        b_idx, h = divmod(core, 2)
        chunks = chunks_by_batch[b_idx]
        in_maps.append({
            "xtq": chunks,
            "xtk": np.ascontiguousarray(chunks[4 * h:4 * h + 4]),
            **w_maps,
        })

    res = run_bass_kernel_spmd(
        _get_nc(), in_maps, core_ids=list(range(N_CORES)), trace=False
    )

    out = np.empty((B, S, D), np.float32)
    for core in range(N_CORES):
        b_idx, h = divmod(core, 2)
        out[b_idx, h * SH:(h + 1) * SH, :] = (
            res.results[core]["out"].astype(np.float32).T
        )
    return out


# revision 12
# speedup vs baseline: 1.4359x; 1.4359x over previous
"""Distributed Trainium2 Bass kernel for single-head attention with
softmax over the QUERY axis (faithful to the reference).

Reference math (per batch b):
    q = x @ Wq + bq          # [S, D]   S=4096, D=48
    k = x @ Wk + bk
    v = x @ Wv + bv
    s = (q @ k.T) / sqrt(D)  # [S_q, S_k]
    p = softmax(s, axis=QUERY)          # normalize each k-COLUMN over q
    out = p @ v              # [S_q, D]

Sharding (k-split): 8 cores = 4 batches x 2 KEY-halves. Core c handles
batch c//2 and key rows [ (c%2)*2048, (c%2+1)*2048 ), for ALL 4096
queries. The softmax denominator colsum[k] = sum_q exp(s[q,k]) is then
fully LOCAL (free-axis accumulation inside the exp instructions) -- no
mid-stream collectives. The only collective is one bf16 ReduceScatter
of the output partials at the end; a tiny dummy collective at t=0
absorbs the cold CC-stream setup.

Layout: everything is computed TRANSPOSED on chip.
  - scores_T[k, q] tiles have k on partitions / q on the free axis, so
    colsum[k] falls out of the exp accum_out (the ACCUMULATOR_READ is
    pipelined behind the next ACTIVATE -- measured free).
  - Q and K are projected with weights DUPLICATED at PE columns 64-111,
    so the row-packing replicas (partitions 64-111) come out of the
    matmul itself -- zero replica DMAs, one-op epilogues.
  - The exp stream: first half sweeps q in two 1024-wide passes over
    all 16 k-tiles (q-major, DMA friendly); second half goes k-major
    (both remaining q-sweeps per tile back to back) so each tile's
    colsum completes immediately and its attention matmuls follow one
    slot later, inside the exp stream.
  - The per-column normalization folds into V (vs[k,:] = v[k,:]/colsum);
    V^T -> V transposes ride the DMA xbar engine.
  - Projections borrow the attention accumulator's PSUM banks before
    attention starts.

SPMD note: one NEFF for all 8 cores, so every AP offset is shared.
Queries live in GLOBAL positions; only the key half differs, via a
host-sliced second input (xtk = the core's own x^T chunks).

bq is dropped: softmax over q is invariant to per-k constant shifts,
and bq only contributes bq.(x_k Wk + bk), constant along q.

exp() runs without max-subtraction: scores*scale is N(0,~1/9), bounded
by ~|2.5| for these inputs, so exp stays well inside fp32 range.
"""

import sys

for _p in ("/opt/trn_rl_repo",):
    if _p not in sys.path:
        sys.path.insert(0, _p)

import numpy as np
import ml_dtypes

import concourse.bass as bass
import concourse.tile as tile
from concourse import bacc, mybir
from concourse.bass_utils import run_bass_kernel_spmd

N_CORES = 8
B = 4
S = 4096
DIM = 768
D = 48
SH = S // 2          # key rows per core / q rows per RS shard
P = 128
NKT = SH // P        # 16 local k-tiles
NC = DIM // P        # 6 contraction tiles for projections
QC = 512             # q chunk = one PSUM bank / projection granularity
NQC = S // QC        # 8 q chunks
SCALE = 1.0 / np.sqrt(np.float32(D))
RG = [[0, 1], [2, 3], [4, 5], [6, 7]]
N_WARM_MM = 56       # PE warmup matmuls during the first DMA wait
                     # (~3.4us cold run flips HAM to 8/8, rest runs warm)

BF16 = mybir.dt.bfloat16
F32 = mybir.dt.float32


def _build():
    nc = bacc.Bacc(
        "TRN2",
        target_bir_lowering=False,
        debug=False,
        num_devices=N_CORES,
    )

    # x^T chunks packed host-side as [chunk][partition][ci][512] so each
    # chunk DMA is 128 descriptors x 6KB (near-peak HBM read).
    xtq_d = nc.dram_tensor("xtq", [NQC, P, NC, QC], BF16, kind="ExternalInput")
    xtk_d = nc.dram_tensor("xtk", [4, P, NC, QC], BF16, kind="ExternalInput")
    # Q / K duplicated at cols 64-111 (row-packing replicas for free).
    wqq_d = nc.dram_tensor("wqq", [P, NC, 112], BF16, kind="ExternalInput")
    wkk_d = nc.dram_tensor("wkk", [P, NC, 112], BF16, kind="ExternalInput")
    wv_d = nc.dram_tensor("wv", [P, NC, 112], BF16, kind="ExternalInput")
    # col 0: K bias duplicated (bk,0,bk); col 1: V bias (0,0,bv)
    bias_d = nc.dram_tensor("bias", [112, 2], F32, kind="ExternalInput")
    out_d = nc.dram_tensor("out", [D, SH], BF16, kind="ExternalOutput")

    with tile.TileContext(nc) as tc:
        with (
            tc.tile_pool(name="consts", bufs=1) as consts,
            tc.tile_pool(name="big", bufs=1) as big,
            tc.tile_pool(name="xtqp", bufs=3) as xtqp,
            tc.tile_pool(name="xtkp", bufs=2) as xtkp,
            tc.tile_pool(name="pop", bufs=1, space="PSUM") as pop,
            tc.tile_pool(name="scp", bufs=2, space="PSUM") as scp,
            tc.tile_pool(name="dram", bufs=1, space="DRAM") as dram,
        ):
            # ---- constants (wqq first: it feeds the PE warmup) -------
            wqq_sb = consts.tile([P, NC, 112], BF16, tag="wqq")
            nc.sync.dma_start(out=wqq_sb, in_=wqq_d[:, :, :])
            wkk_sb = consts.tile([P, NC, 112], BF16, tag="wkk")
            nc.sync.dma_start(out=wkk_sb, in_=wkk_d[:, :, :])
            wv_sb = consts.tile([P, NC, 112], BF16, tag="wv")
            nc.sync.dma_start(out=wv_sb, in_=wv_d[:, :, :])
            bias_sb = consts.tile([112, 2], F32, tag="bias")
            nc.sync.dma_start(out=bias_sb, in_=bias_d[:, :])

            # vT rows 112:127 are junk fed to the xbar transpose; zero
            # them so v_sb cols 48:63 are defined (never read). On the
            # DVE queue BEFORE any epilogue: the gpsimd queue is frozen
            # by the collective prelude barrier for the first ~70us.
            vT = big.tile([P, SH], BF16, tag="vT")       # V^T @64-111
            nc.vector.memset(vT[96:P, :], 0)

            # NOTE: no CC-stream warmup collective. Tile recycles
            # completion semaphores, so ANY early collective makes some
            # later instruction (whichever reuses its semaphore) wait on
            # the CC setup barrier -- measured 19-113us of variable
            # latency injected mid-stream. The setup barrier runs
            # concurrently at kernel start on the CC stream regardless,
            # so the final ReduceScatter only pays first-op overhead.

            # ---- persistent SBUF tensors -----------------------------
            qT = big.tile([112, S], BF16, tag="qT")      # Q^T @0-47 and @64-111
            kkT = big.tile([112, SH], BF16, tag="kkT")   # K^T @0-47 and @64-111
            v_sb = big.tile([P, NKT, 64], BF16, tag="v")     # V [k, d] tiles
            vs_sb = big.tile([P, NKT, D], BF16, tag="vs")    # V / colsum
            e_sb = big.tile([P, NKT, S], BF16, tag="e")      # exp(scores^T)
            csh = big.tile([P, NKT, 4], F32, tag="csh")      # colsum partials
            cs = big.tile([P, NKT], F32, tag="cs")
            rec = big.tile([P, NKT], F32, tag="rec")
            out_sb = big.tile([P, 4, QC], BF16, tag="out")

            # attention accumulator: 4 banks, column-packed
            #   parts 0:48  bank b = global q-chunk b     (q 0..2047)
            #   parts 64:112 bank b = global q-chunk 4+b  (q 2048..4095)
            # projections borrow bank slots 0/1 BEFORE attention starts.
            po = pop.tile([P, 4, QC], F32, tag="po")

            # ---- PE warmup: HAM to 8/8 during the first DMA wait -----
            warm_ps = scp.tile([P, 2, QC], F32, tag="sct")
            for _ in range(N_WARM_MM):
                nc.tensor.matmul(
                    warm_ps[0:112, 0, 0:112], wqq_sb[:, 0, :], wqq_sb[:, 0, :],
                    start=True, stop=True, skip_group_check=True,
                )

            # ---- projections into rotating po bank slots -------------
            qdma = {}
            kdma = {}
            slot_rr = [0]

            def q_dma(c, split=False):
                t = xtqp.tile([P, NC, QC], BF16, tag="xtq")
                if split:
                    nc.sync.dma_start(out=t[:, 0:3, :], in_=xtq_d[c][:, 0:3, :])
                    nc.sync.dma_start(out=t[:, 3:6, :], in_=xtq_d[c][:, 3:6, :])
                else:
                    nc.sync.dma_start(out=t, in_=xtq_d[c])
                qdma[c] = t

            def k_dma(j, split=False):
                t = xtkp.tile([P, NC, QC], BF16, tag="xtk")
                if split:
                    nc.sync.dma_start(out=t[:, 0:3, :], in_=xtk_d[j][:, 0:3, :])
                    nc.sync.dma_start(out=t[:, 3:6, :], in_=xtk_d[j][:, 3:6, :])
                else:
                    nc.sync.dma_start(out=t, in_=xtk_d[j])
                kdma[j] = t

            def _proj(t, w_sb, dst, bias_ap):
                sl = slot_rr[0]
                slot_rr[0] ^= 1
                pp = po[0:112, sl, :]
                for ci in range(NC):
                    nc.tensor.matmul(
                        pp, w_sb[:, ci, :], t[:, ci, :],
                        start=(ci == 0), stop=(ci == NC - 1),
                        skip_group_check=True,
                    )
                if bias_ap is None:
                    nc.vector.tensor_copy(out=dst, in_=pp)
                else:
                    nc.vector.tensor_scalar(
                        out=dst, in0=pp, scalar1=bias_ap, scalar2=None,
                        op0=mybir.AluOpType.add,
                    )

            def q_proj(c):
                t = qdma[c]
                qsl = slice(c * QC, (c + 1) * QC)
                _proj(t, wqq_sb, qT[0:112, qsl], None)
                qdma.pop(c)

            def k_proj(j):
                t = kdma[j]
                ksl = slice(j * QC, (j + 1) * QC)
                _proj(t, wkk_sb, kkT[0:112, ksl], bias_sb[0:112, 0:1])

            def v_proj(j):
                t = kdma.pop(j)
                ksl = slice(j * QC, (j + 1) * QC)
                sl = slot_rr[0]
                slot_rr[0] ^= 1
                pp = po[0:112, sl, :]
                for ci in range(NC):
                    nc.tensor.matmul(
                        pp, wv_sb[:, ci, :], t[:, ci, :],
                        start=(ci == 0), stop=(ci == NC - 1),
                        skip_group_check=True,
                    )
                nc.vector.tensor_scalar(
                    out=vT[64:112, ksl], in0=pp[64:112, :],
                    scalar1=bias_sb[64:112, 1:2], scalar2=None,
                    op0=mybir.AluOpType.add,
                )

            def v_tr(kt):
                # V^T [64,128] slice -> v_sb [128,64] via DMA xbar
                nc.sync.dma_start_transpose(
                    out=v_sb[:, kt, :],
                    in_=vT[64:P, kt * P:(kt + 1) * P],
                )

            # ---- scores + exp for one (k-tile, sweep) unit -----------
            def unit(kt, sw):
                sct = scp.tile([P, 2, QC], F32, tag="sct")
                ksl = slice(kt * P, (kt + 1) * P)
                nc.tensor.matmul(
                    sct[:, 0, :],
                    kkT[0:D, ksl],
                    qT[0:D, (2 * sw) * QC:(2 * sw + 1) * QC],
                    start=True, stop=True,
                    tile_position=(0, 0), skip_group_check=True,
                )
                nc.tensor.matmul(
                    sct[:, 1, :],
                    kkT[64:64 + D, ksl],
                    qT[64:64 + D, (2 * sw + 1) * QC:(2 * sw + 2) * QC],
                    start=True, stop=True,
                    tile_position=(64, 0), skip_group_check=True,
                )
                nc.scalar.activation(
                    out=e_sb[:, kt, sw * 2 * QC:(sw + 1) * 2 * QC],
                    in_=sct[:, :, :],
                    func=mybir.ActivationFunctionType.Exp,
                    scale=float(SCALE),
                    accum_out=csh[:, kt, sw:sw + 1],
                )

            # ---- colsum finish + V scaling for one k-tile ------------
            def finish(kt):
                nc.vector.tensor_reduce(
                    out=cs[:, kt:kt + 1], in_=csh[:, kt, :],
                    axis=mybir.AxisListType.X, op=mybir.AluOpType.add,
                )
                nc.vector.reciprocal(out=rec[:, kt:kt + 1], in_=cs[:, kt:kt + 1])
                nc.vector.tensor_scalar(
                    out=vs_sb[:, kt, :], in0=v_sb[:, kt, 0:D],
                    scalar1=rec[:, kt:kt + 1], scalar2=None,
                    op0=mybir.AluOpType.mult,
                )

            # ---- attention pair-halves (column-packed) ---------------
            def attn(kt, half):
                first = kt == 0
                last = kt == NKT - 1
                for p in (0, 1) if half == 0 else (2, 3):
                    nc.tensor.matmul(
                        po[0:D, p, :],
                        vs_sb[:, kt, :],
                        e_sb[:, kt, p * QC:(p + 1) * QC],
                        start=first, stop=last,
                        tile_position=(0, 0), skip_group_check=True,
                    )
                    pb = (p + 1) % 4
                    nc.tensor.matmul(
                        po[64:64 + D, pb, :],
                        vs_sb[:, kt, :],
                        e_sb[:, kt, (4 + pb) * QC:(5 + pb) * QC],
                        start=first, stop=last,
                        tile_position=(0, 64), skip_group_check=True,
                    )

            # ================= emission schedule ======================
            # Front: chunk 0 of K + Q chunks 0,1 gate the first unit.
            k_dma(0, split=True)
            q_dma(0, split=True)
            q_dma(1, split=True)
            k_proj(0)
            q_proj(0)
            q_proj(1)
            k_dma(1)
            q_dma(2)

            # Fill work interleaved into the first-half exp stream.
            # Input DMA issues all precede the v_tr transposes on the
            # sync queue so the (slow) transpose issues never head-block
            # a chunk load.
            fill = [
                lambda: k_proj(1),       # k chunk 1 ready before slot 4
                lambda: v_proj(0),       # frees xtk buf 0
                lambda: k_dma(2),
                lambda: q_dma(3),
                lambda: q_proj(2),
                lambda: k_proj(2),       # ready before slot 8
                lambda: v_proj(1),       # frees xtk buf 1
                lambda: k_dma(3),
                lambda: q_proj(3),
                lambda: k_proj(3),       # ready before slot 12
                lambda: q_dma(4),
                lambda: v_proj(2),
                lambda: q_proj(4),
                lambda: q_dma(5),
                lambda: v_proj(3),
                lambda: q_proj(5),
                lambda: q_dma(6),
                lambda: q_proj(6),
                lambda: q_dma(7),
                lambda: q_proj(7),
            ] + [(lambda kt=kt: v_tr(kt)) for kt in range(NKT)]
            fi = [0]

            def pop_fill(n):
                for _ in range(n):
                    if fi[0] < len(fill):
                        fill[fi[0]]()
                        fi[0] += 1

            # First half: sweeps 0,1 q-major.
            for sw in range(2):
                for kt in range(NKT):
                    unit(kt, sw)
                    pop_fill(1 if (sw == 0 or kt < 4) else 2)

            # Second half: k-major, attention one tile behind.
            for kt in range(NKT):
                unit(kt, 2)
                if kt > 0:
                    finish(kt - 1)
                    attn(kt - 1, 0)
                unit(kt, 3)
                if kt > 0:
                    attn(kt - 1, 1)
            finish(NKT - 1)
            attn(NKT - 1, 0)
            attn(NKT - 1, 1)

            # ---- epilogue: PSUM -> SBUF(bf16) -> DRAM -> RS ----------
            nc.vector.tensor_copy(out=out_sb[:, 0:2, :], in_=po[:, 0:2, :])
            nc.scalar.copy(out=out_sb[:, 2:4, :], in_=po[:, 2:4, :])

            cc_in = dram.tile([2, D, SH], BF16, tag="cc_in")
            cc_out = dram.tile([D, SH], BF16, tag="cc_out")
            nc.sync.dma_start(
                out=cc_in[0],
                in_=out_sb[0:D, :, :].rearrange("p a b -> p (a b)"),
            )
            nc.scalar.dma_start(
                out=cc_in[1],
                in_=out_sb[64:64 + D, :, :].rearrange("p a b -> p (a b)"),
            )
            nc.gpsimd.collective_compute(
                "ReduceScatter",
                mybir.AluOpType.add,
                replica_groups=RG,
                ins=[cc_in.opt()],
                outs=[cc_out.opt()],
            )
            nc.sync.dma_start(out=out_d[:, :], in_=cc_out)

    nc.compile()
    return nc


_NC_CACHE = None


def _get_nc():
    global _NC_CACHE
    if _NC_CACHE is None:
        _NC_CACHE = _build()
    return _NC_CACHE


def kernel(x, Wq, bq, Wk, bk, Wv, bv):
    x = np.asarray(x, np.float32)
    bf = ml_dtypes.bfloat16

    def pack_dup(W):
        """[768, 48] -> [128, 6, 112] bf16 with cols duplicated @64."""
        full = np.zeros((DIM, 112), np.float32)
        full[:, 0:D] = W
        full[:, 64:64 + D] = W
        return np.ascontiguousarray(
            full.reshape(NC, P, 112).transpose(1, 0, 2)
        ).astype(bf)

    wqq_h = pack_dup(np.asarray(Wq, np.float32))
    wkk_h = pack_dup(np.asarray(Wk, np.float32))
    wv_full = np.zeros((DIM, 112), np.float32)
    wv_full[:, 64:64 + D] = np.asarray(Wv, np.float32)
    wv_h = np.ascontiguousarray(
        wv_full.reshape(NC, P, 112).transpose(1, 0, 2)
    ).astype(bf)
    bias_h = np.zeros((112, 2), np.float32)
    bias_h[0:D, 0] = np.asarray(bk, np.float32).ravel()
    bias_h[64:64 + D, 0] = np.asarray(bk, np.float32).ravel()
    bias_h[64:64 + D, 1] = np.asarray(bv, np.float32).ravel()
    # bq is mathematically irrelevant: softmax over the query axis is
    # invariant to per-key constant shifts.

    w_maps = {"wqq": wqq_h, "wkk": wkk_h, "wv": wv_h, "bias": bias_h}

    in_maps = []
    chunks_by_batch = []
    for b_idx in range(B):
        xT = np.ascontiguousarray(x[b_idx].T)                  # [768, 4096]
        chunks = np.ascontiguousarray(
            xT.reshape(NC, P, NQC, QC).transpose(2, 1, 0, 3)
        ).astype(bf)                                           # [8,128,6,512]
        chunks_by_batch.append(chunks)
    for core in range(N_CORES):
        b_idx, h = divmod(core, 2)
        chunks = chunks_by_batch[b_idx]
        in_maps.append({
            "xtq": chunks,
            "xtk": np.ascontiguousarray(chunks[4 * h:4 * h + 4]),
            **w_maps,
        })

    res = run_bass_kernel_spmd(
        _get_nc(), in_maps, core_ids=list(range(N_CORES)), trace=False
    )

    out = np.empty((B, S, D), np.float32)
    for core in range(N_CORES):
        b_idx, h = divmod(core, 2)
        out[b_idx, h * SH:(h + 1) * SH, :] = (
            res.results[core]["out"].astype(np.float32).T
        )
    return out
